# revision 1
# baseline (speedup 1.0000x reference)
"""Trainium2 Bass kernel for the ConvolutionalCapsule module.

Sharding: data-parallel over (batch, H-half): core k handles b = k//2,
output rows h in [6*(k%2), 6*(k%2)+6), i.e. 72 spatial positions per core.
Weights are replicated. All FLOPs run on-device; the host only does
layout/gather of inputs (patch extraction + weight transposes).

Device algorithm per core (pos = 72 positions):
  preds[f,c,o] = sum_i W[f,c,o,i] * P[c,i] is never materialized. Instead:
  iter0:  out0 = squash(mean_c preds) via one K=(c,i)=2304 matmul.
  iter1:  agr[f,c]  = sum_o preds * out0  computed as
             V[f,(i,c)] = sum_o W * out0   (per-f K=16 matmuls, row-tiled)
             VP = V  *  P                  (elementwise, fp16 DVE)
             agr = sum_i VP                (identity-lhsT matmuls, PSUM accum)
          cc = softmax_f(agr) folded as  e=exp(agr), P' = P/Z,
          centroids = sum_{(i,c)} (e * P') * W   (per-f K=(i,c) matmuls)
          out = squash(centroids)
Layouts keep (i, c-block-of-128) on SBUF partitions for everything after
out0, so the f-softmax reductions run along the free axis.
"""
import numpy as np

KH = KW = 3
B, H, WD, FIN, DIN = 4, 14, 14, 32, 8
F, C, DO, DI = 32, 288, 16, 8
NPOS = 72
CBLK = 3
NCHUNK = DI * CBLK  # 24
EPS = 1e-7

_CACHE: dict = {}


def _chunk_rows(t):
    i, cb = divmod(t, CBLK)
    c0 = cb * 128
    return i, c0, min(128, C - c0)


def _host_weights(Wm):
    """Wm: [F, C, DO, DI] float32 -> device weight layouts (fp16)."""
    w_r = np.zeros((NCHUNK, 128, F * DO), np.float16)
    for t in range(NCHUNK):
        i, c0, n = _chunk_rows(t)
        w_r[t, :n, :] = (
            Wm[:, c0:c0 + n, :, i].transpose(1, 0, 2).reshape(n, F * DO)
        )
    w_vt = np.zeros((4, 32, 8, NCHUNK, 128), np.float16)
    for f in range(F):
        g, j = divmod(f, 4)
        for t in range(NCHUNK):
            i, c0, n = _chunk_rows(t)
            w_vt[j, :DO, g, t, :n] = Wm[f, c0:c0 + n, :, i].T
    w_r = w_r.transpose(1, 0, 2).reshape(128, NCHUNK * F * DO).copy()
    return w_r, w_vt.reshape(128, 8 * NCHUNK * 128)


def _host_patches(x, k):
    """Patch tensor for core k in (i, c-block) chunk layout: [24, 128, 72]."""
    b, hh = divmod(k, 2)
    h0 = 6 * hh
    # P[pos, c=(kh,kw,fin), i]
    P = np.empty((6, 12, KH, KW, FIN, DIN), np.float32)
    for kh in range(KH):
        for kw in range(KW):
            for h in range(6):
                P[h, :, kh, kw] = x[b, h0 + h + kh, kw:kw + 12]
    P = P.reshape(NPOS, C, DIN)
    p_ct = np.zeros((NCHUNK, 128, NPOS), np.float16)
    for t in range(NCHUNK):
        i, c0, n = _chunk_rows(t)
        p_ct[t, :n, :] = P[:, c0:c0 + n, i].T
    return p_ct.transpose(1, 0, 2).reshape(128, NCHUNK * NPOS).copy()


def _build():
    import concourse.bass as bass
    import concourse.bacc as bacc
    import concourse.mybir as mybir
    import concourse.tile as tile

    F16, F32 = mybir.dt.float16, mybir.dt.float32
    AX = mybir.AxisListType
    AF = mybir.ActivationFunctionType

    nc = bacc.Bacc(None, target_bir_lowering=False, debug=False)

    p_ct_d = nc.dram_tensor("p_ct", [128, NCHUNK * NPOS], F16, kind="ExternalInput")
    w_r_d = nc.dram_tensor("w_r", [128, NCHUNK * F * DO], F16, kind="ExternalInput")
    w_vt_d = nc.dram_tensor("w_vt", [128, 8 * NCHUNK * 128], F16, kind="ExternalInput")
    eye72_d = nc.dram_tensor("eye72", [NPOS, NPOS], F32, kind="ExternalInput")
    eye128h_d = nc.dram_tensor("eye128h", [128, 128], F16, kind="ExternalInput")
    eye128f_d = nc.dram_tensor("eye128f", [128, 128], F32, kind="ExternalInput")
    y_d = nc.dram_tensor("y", [NPOS, F * DO], F32, kind="ExternalOutput")

    with tile.TileContext(nc) as tc:
        with (
            tc.tile_pool(name="const", bufs=1) as const,
            tc.tile_pool(name="work", bufs=1) as work,
            tc.tile_pool(name="ring", bufs=2) as ring,
            tc.tile_pool(name="acc", bufs=3, space=bass.MemorySpace.PSUM) as acc,
            tc.tile_pool(name="tps", bufs=2, space=bass.MemorySpace.PSUM) as tps,
            tc.tile_pool(name="cenp", bufs=2, space=bass.MemorySpace.PSUM) as cenp,
        ):
            # ---------------- loads ----------------
            p_ct = const.tile([128, NCHUNK * NPOS], F16, tag="p_ct")
            nc.sync.dma_start(p_ct[:], p_ct_d[:])
            w_r = const.tile([128, NCHUNK * F * DO], F16, tag="w_r")
            nc.sync.dma_start(w_r[:], w_r_d[:])
            w_vt = const.tile([128, 8 * NCHUNK * 128], F16, tag="w_vt")
            nc.sync.dma_start(w_vt[:], w_vt_d[:])
            eye72 = const.tile([NPOS, NPOS], F32, tag="eye72")
            nc.sync.dma_start(eye72[:], eye72_d[:])
            eye128h = const.tile([128, 128], F16, tag="eye128h")
            nc.sync.dma_start(eye128h[:], eye128h_d[:])
            eye128f = const.tile([128, 128], F32, tag="eye128f")
            nc.sync.dma_start(eye128f[:], eye128f_d[:])

            def squash(src_ap, dst_ap, pre_scale, tag):
                """dst = squash(src * pre_scale) ; src/dst free = (f,o)=512."""
                s = work.tile([NPOS, F * DO], F32, tag=f"{tag}_s")
                nc.scalar.activation(s[:], src_ap, AF.Copy, scale=pre_scale)
                sq = work.tile([NPOS, F * DO], F32, tag=f"{tag}_sq")
                nc.scalar.activation(sq[:], s[:], AF.Square)
                sn = work.tile([NPOS, F], F32, tag=f"{tag}_sn")
                nc.vector.reduce_sum(
                    sn[:], sq[:].rearrange("p (f o) -> p f o", o=DO), axis=AX.X
                )
                t1 = work.tile([NPOS, F], F32, tag=f"{tag}_t1")
                nc.vector.tensor_scalar_add(t1[:], sn[:], 1.0)
                r1 = work.tile([NPOS, F], F32, tag=f"{tag}_r1")
                nc.vector.reciprocal(r1[:], t1[:])
                se = work.tile([NPOS, F], F32, tag=f"{tag}_se")
                nc.vector.tensor_scalar_add(se[:], sn[:], EPS)
                r2 = work.tile([NPOS, F], F32, tag=f"{tag}_r2")
                nc.scalar.activation(r2[:], se[:], AF.Sqrt)
                r3 = work.tile([NPOS, F], F32, tag=f"{tag}_r3")
                nc.vector.reciprocal(r3[:], r2[:])
                sc = work.tile([NPOS, F], F32, tag=f"{tag}_sc")
                nc.vector.tensor_mul(sc[:], sn[:], r1[:])
                sc2 = work.tile([NPOS, F], F32, tag=f"{tag}_sc2")
                nc.vector.tensor_mul(sc2[:], sc[:], r3[:])
                bc = sc2[:].unsqueeze(2).broadcast_to((NPOS, F, DO))
                nc.vector.tensor_mul(
                    dst_ap, s[:].rearrange("p (f o) -> p f o", o=DO), bc
                )

            # ---------------- stage B: out0 ----------------
            o0p = acc.tile([NPOS, F * DO], F32, tag="mm")
            for t in range(NCHUNK):
                nc.tensor.matmul(
                    o0p[:],
                    p_ct[:, t * NPOS:(t + 1) * NPOS],
                    w_r[:, t * F * DO:(t + 1) * F * DO],
                    start=(t == 0),
                    stop=(t == NCHUNK - 1),
                )
            out0_pad = work.tile([NPOS, F * 32], F32, tag="out0_pad")
            nc.vector.memset(out0_pad[:], 0.0)
            squash(
                o0p[:],
                out0_pad[:].rearrange("p (f s) -> p f s", s=32)[:, :, 0:DO],
                1.0 / F,
                "sq1",
            )
            # transposes -> out0T [128 = (j,o-slot), 8*72]
            out0T = work.tile([128, 8 * NPOS], F16, tag="out0T")
            for g in range(8):
                tp = tps.tile([128, 128], F32, tag="tp")
                nc.tensor.transpose(
                    tp[:, 0:NPOS], out0_pad[:, g * 128:(g + 1) * 128], eye72[:]
                )
                nc.scalar.copy(out0T[:, g * NPOS:(g + 1) * NPOS], tp[:, 0:NPOS])

            # ---------------- stage D: V, VP, agreement ----------------
            agr = work.tile([128, F * CBLK * NPOS], F16, tag="agr")
            for f in range(F):
                g, j = divmod(f, 4)
                V = ring.tile([128, NCHUNK * NPOS], F16, tag="V")
                for q in range(4):
                    nq = 7 if q < 3 else 3
                    vq = acc.tile([128, F * DO], F32, tag="mm")
                    for u in range(nq):
                        t = 7 * q + u
                        nc.tensor.matmul(
                            vq[:, u * NPOS:(u + 1) * NPOS],
                            w_vt[32 * j:32 * (j + 1),
                                 (g * NCHUNK + t) * 128:(g * NCHUNK + t + 1) * 128],
                            out0T[32 * j:32 * (j + 1), g * NPOS:(g + 1) * NPOS],
                            start=True,
                            stop=True,
                            tile_position=(32 * j, 0),
                        )
                    nc.scalar.copy(
                        V[:, q * 7 * NPOS:q * 7 * NPOS + nq * NPOS],
                        vq[:, 0:nq * NPOS],
                    )
                VP = ring.tile([128, NCHUNK * NPOS], F16, tag="VP")
                nc.vector.tensor_mul(VP[:], V[:], p_ct[:])
                agp = acc.tile([128, CBLK * NPOS], F32, tag="mm")
                for cb in range(CBLK):
                    for i in range(DI):
                        t = i * CBLK + cb
                        nc.tensor.matmul(
                            agp[:, cb * NPOS:(cb + 1) * NPOS],
                            eye128h[:],
                            VP[:, t * NPOS:(t + 1) * NPOS],
                            start=(i == 0),
                            stop=(i == DI - 1),
                        )
                nc.scalar.copy(
                    agr[:, f * CBLK * NPOS:(f + 1) * CBLK * NPOS], agp[:]
                )

            # ---------------- softmax pieces ----------------
            e = work.tile([128, F * CBLK * NPOS], F16, tag="e")
            nc.scalar.activation(e[:], agr[:], AF.Exp)
            Zf = work.tile([128, CBLK * NPOS], F32, tag="Zf")
            nc.vector.reduce_sum(
                Zf[:],
                e[:].rearrange("p (f cb n) -> p cb n f", f=F, cb=CBLK),
                axis=AX.X,
            )
            Zr = work.tile([128, CBLK * NPOS], F32, tag="Zr")
            nc.vector.reciprocal(Zr[:], Zf[:])
            pp = work.tile([128, NCHUNK * NPOS], F16, tag="pp")
            nc.vector.tensor_mul(
                pp[:].rearrange("p (i cb n) -> p i cb n", i=DI, cb=CBLK),
                p_ct[:].rearrange("p (i cb n) -> p i cb n", i=DI, cb=CBLK),
                Zr[:].rearrange("p (cb n) -> p cb n", cb=CBLK)
                .unsqueeze(1)
                .broadcast_to((128, DI, CBLK, NPOS)),
            )

            # ---------------- stage S + centroids ----------------
            cen_sb = work.tile([128, 8 * NPOS], F32, tag="cen_sb")
            for g in range(8):
                cg = cenp.tile([128, NPOS], F32, tag="cen")
                nc.vector.memset(cg[:], 0.0)
                for j in range(4):
                    f = 4 * g + j
                    S = ring.tile([128, NCHUNK * NPOS], F16, tag="S")
                    nc.vector.tensor_mul(
                        S[:].rearrange("p (i cb n) -> p i cb n", i=DI, cb=CBLK),
                        pp[:].rearrange("p (i cb n) -> p i cb n", i=DI, cb=CBLK),
                        e[:, f * CBLK * NPOS:(f + 1) * CBLK * NPOS]
                        .rearrange("p (cb n) -> p cb n", cb=CBLK)
                        .unsqueeze(1)
                        .broadcast_to((128, DI, CBLK, NPOS)),
                    )
                    for t in range(NCHUNK):
                        nc.tensor.matmul(
                            cg[32 * j:32 * j + DO, :],
                            w_r[:, t * F * DO + f * DO:t * F * DO + (f + 1) * DO],
                            S[:, t * NPOS:(t + 1) * NPOS],
                            start=(t == 0),
                            stop=(t == NCHUNK - 1),
                            tile_position=(0, 32 * j),
                        )
                nc.scalar.copy(cen_sb[:, g * NPOS:(g + 1) * NPOS], cg[:])

            # ---------------- squash2 + output ----------------
            opre = work.tile([NPOS, 8 * 128], F32, tag="opre")
            for g in range(8):
                tp2 = tps.tile([128, 128], F32, tag="tp")
                nc.tensor.transpose(
                    tp2[0:NPOS, :], cen_sb[:, g * NPOS:(g + 1) * NPOS], eye128f[:]
                )
                nc.scalar.copy(opre[:, g * 128:(g + 1) * 128], tp2[0:NPOS, :])
            gat = work.tile([NPOS, F * DO], F32, tag="gat")
            nc.vector.tensor_copy(
                gat[:].rearrange("p (g j o) -> p g j o", g=8, j=4),
                opre[:].rearrange("p (g j s) -> p g j s", g=8, j=4)[:, :, :, 0:DO],
            )
            y_sb = work.tile([NPOS, F * DO], F32, tag="y_sb")
            squash(gat[:], y_sb[:].rearrange("p (f o) -> p f o", o=DO), 1.0, "sq2")
            nc.sync.dma_start(y_d[:], y_sb[:])

    nc.compile()
    return nc


def _get_program():
    if "nc" not in _CACHE:
        _CACHE["nc"] = _build()
    return _CACHE["nc"]


def kernel(x, W):
    from concourse.bass_utils import run_bass_kernel_spmd

    x = np.asarray(x, np.float32)
    Wm = np.asarray(W, np.float32)[0, 0, 0]
    nc = _get_program()

    w_r, w_vt = _host_weights(Wm)
    eye72 = np.eye(NPOS, dtype=np.float32)
    eye128h = np.eye(128, dtype=np.float16)
    eye128f = np.eye(128, dtype=np.float32)
    in_maps = []
    for k in range(8):
        in_maps.append({
            "p_ct": _host_patches(x, k),
            "w_r": w_r,
            "w_vt": w_vt,
            "eye72": eye72,
            "eye128h": eye128h,
            "eye128f": eye128f,
        })
    res = run_bass_kernel_spmd(nc, in_maps, list(range(8)))
    Ho, Wo = H - KH + 1, WD - KW + 1
    y = np.empty((B, Ho, Wo, F, DO), np.float32)
    for k in range(8):
        b, hh = divmod(k, 2)
        y[b, 6 * hh:6 * hh + 6] = res.results[k]["y"].reshape(6, Wo, F, DO)
    return y



# revision 5
# speedup vs baseline: 1.4439x; 1.4439x over previous
"""Trainium2 Bass kernel for the ConvolutionalCapsule module.

Sharding: data-parallel over (batch, H-half): core k handles b = k//2,
output rows h in [6*(k%2), 6*(k%2)+6), i.e. 72 spatial positions per core.
Weights are replicated. All FLOPs run on-device; the host only does
layout/gather of inputs (patch extraction + weight transposes).

Device algorithm per core (pos = 72 positions), chunk layout keeps
(c-block-of-128) on SBUF partitions, chunks indexed t = i*3 + cb:
  iter0:  out0 = squash(mean_c preds) via one K=(c,i)=2304 matmul.
  iter1:  V[f,c,i] = sum_o W[f,c,o,i]*out0[f,o] via W-stationary matmuls
            [(4f,32)=128, c-chunk] streaming a block-diagonal out0
            (8 quartet tiles [128, 4f*72], built once, 32-aligned blocks).
          VP = V (.) P  (PSUM exit fused / split scalar+vector)
          agr[c, (f,pos)] = sum_i VP  (identity-matmul PSUM accumulation)
          e = exp(agr)  (scalar engine, fused with PSUM exit)
          Z = sum_f e ; pp = P * (1/Z)
          S_f = e_f (.) pp ; centroids via col-tiled w_r matmuls
          out = squash(centroids)
"""
import numpy as np

KH = KW = 3
B, H, WD, FIN, DIN = 4, 14, 14, 32, 8
F, C, DO, DI = 32, 288, 16, 8
NPOS = 72
CBLK = 3
NCHUNK = DI * CBLK  # 24
NQRT = 8            # f-quartets of 4
EPS = 1e-7

_CACHE: dict = {}


def _chunk_rows(t):
    i, cb = divmod(t, CBLK)
    c0 = cb * 128
    return i, c0, min(128, C - c0)


def _host_weights(Wm):
    """Wm: [F, C, DO, DI] float32 -> device weight layouts (fp16)."""
    w_r = np.zeros((NCHUNK, 128, F * DO), np.float16)
    for t in range(NCHUNK):
        i, c0, n = _chunk_rows(t)
        w_r[t, :n, :] = (
            Wm[:, c0:c0 + n, :, i].transpose(1, 0, 2).reshape(n, F * DO)
        )
    w_r = w_r.transpose(1, 0, 2).reshape(128, NCHUNK * F * DO).copy()
    # w8[(l,o-slot32), (Q,t,c)]: stationary V-matmul weights
    w8 = np.zeros((NQRT, NCHUNK, 128, 128), np.float16)
    for Q in range(NQRT):
        for t in range(NCHUNK):
            i, c0, n = _chunk_rows(t)
            for l in range(4):
                f = 4 * Q + l
                w8[Q, t, 32 * l:32 * l + DO, :n] = Wm[f, c0:c0 + n, :, i].T
    w8 = w8.transpose(2, 0, 1, 3).reshape(128, NQRT * NCHUNK * 128).copy()
    return w_r, w8


def _host_patches(x, k):
    """Patch tensor for core k in (i, c-block) chunk layout: [128, 24*72]."""
    b, hh = divmod(k, 2)
    h0 = 6 * hh
    P = np.empty((6, 12, KH, KW, FIN, DIN), np.float32)
    for kh in range(KH):
        for kw in range(KW):
            for h in range(6):
                P[h, :, kh, kw] = x[b, h0 + h + kh, kw:kw + 12]
    P = P.reshape(NPOS, C, DIN)
    p_ct = np.zeros((NCHUNK, 128, NPOS), np.float16)
    for t in range(NCHUNK):
        i, c0, n = _chunk_rows(t)
        p_ct[t, :n, :] = P[:, c0:c0 + n, i].T
    return p_ct.transpose(1, 0, 2).reshape(128, NCHUNK * NPOS).copy()


def _build():
    import concourse.bass as bass
    import concourse.bacc as bacc
    import concourse.mybir as mybir
    import concourse.tile as tile

    F16, F32 = mybir.dt.float16, mybir.dt.float32
    AX = mybir.AxisListType
    AF = mybir.ActivationFunctionType

    nc = bacc.Bacc(None, target_bir_lowering=False, debug=False)

    p_ct_d = nc.dram_tensor("p_ct", [128, NCHUNK * NPOS], F16, kind="ExternalInput")
    w_r_d = nc.dram_tensor("w_r", [128, NCHUNK * F * DO], F16, kind="ExternalInput")
    w8_d = nc.dram_tensor("w8", [128, NQRT * NCHUNK * 128], F16, kind="ExternalInput")
    eye72_d = nc.dram_tensor("eye72", [NPOS, NPOS], F32, kind="ExternalInput")
    eye128h_d = nc.dram_tensor("eye128h", [128, 128], F16, kind="ExternalInput")
    eye128f_d = nc.dram_tensor("eye128f", [128, 128], F32, kind="ExternalInput")
    y_d = nc.dram_tensor("y", [NPOS, F * DO], F32, kind="ExternalOutput")

    QBD = 4 * NPOS            # 288 = 4f * 72pos, one quartet's free extent

    with tile.TileContext(nc) as tc:
        with (
            tc.tile_pool(name="const", bufs=1) as const,
            tc.tile_pool(name="work", bufs=1) as work,
            tc.tile_pool(name="ring", bufs=2) as ring,
            tc.tile_pool(name="vps", bufs=2, space=bass.MemorySpace.PSUM) as vps,
            tc.tile_pool(name="tps", bufs=2, space=bass.MemorySpace.PSUM) as tps,
            tc.tile_pool(name="acc", bufs=2, space=bass.MemorySpace.PSUM) as acc,
            tc.tile_pool(name="cenp", bufs=2, space=bass.MemorySpace.PSUM) as cenp,
        ):
            # ---------------- loads ----------------
            p_ct = const.tile([128, NCHUNK * NPOS], F16, tag="p_ct")
            nc.sync.dma_start(p_ct[:], p_ct_d[:])
            w_r = const.tile([128, NCHUNK * F * DO], F16, tag="w_r")
            nc.sync.dma_start(w_r[:], w_r_d[:])
            w8 = const.tile([128, NQRT * NCHUNK * 128], F16, tag="w8")
            nc.sync.dma_start(w8[:], w8_d[:])
            eye72 = const.tile([NPOS, NPOS], F32, tag="eye72")
            nc.sync.dma_start(eye72[:], eye72_d[:])
            eye128h = const.tile([128, 128], F16, tag="eye128h")
            nc.sync.dma_start(eye128h[:], eye128h_d[:])
            eye128f = const.tile([128, 128], F32, tag="eye128f")
            nc.sync.dma_start(eye128f[:], eye128f_d[:])

            def squash(src_ap, dst_ap, pre_scale, tag):
                """dst = squash(src * pre_scale) ; src free = (f,o)=512."""
                s = work.tile([NPOS, F * DO], F32, tag=f"{tag}_s")
                nc.scalar.activation(s[:], src_ap, AF.Copy, scale=pre_scale)
                sq = work.tile([NPOS, F * DO], F32, tag=f"{tag}_sq")
                nc.scalar.activation(sq[:], s[:], AF.Square)
                sn = work.tile([NPOS, F], F32, tag=f"{tag}_sn")
                nc.vector.reduce_sum(
                    sn[:], sq[:].rearrange("p (f o) -> p f o", o=DO), axis=AX.X
                )
                t1 = work.tile([NPOS, F], F32, tag=f"{tag}_t1")
                nc.vector.tensor_scalar_add(t1[:], sn[:], 1.0)
                r1 = work.tile([NPOS, F], F32, tag=f"{tag}_r1")
                nc.vector.reciprocal(r1[:], t1[:])
                se = work.tile([NPOS, F], F32, tag=f"{tag}_se")
                nc.vector.tensor_scalar_add(se[:], sn[:], EPS)
                r2 = work.tile([NPOS, F], F32, tag=f"{tag}_r2")
                nc.scalar.activation(r2[:], se[:], AF.Sqrt)
                r3 = work.tile([NPOS, F], F32, tag=f"{tag}_r3")
                nc.vector.reciprocal(r3[:], r2[:])
                sc = work.tile([NPOS, F], F32, tag=f"{tag}_sc")
                nc.vector.tensor_mul(sc[:], sn[:], r1[:])
                sc2 = work.tile([NPOS, F], F32, tag=f"{tag}_sc2")
                nc.vector.tensor_mul(sc2[:], sc[:], r3[:])
                bc = sc2[:].unsqueeze(2).broadcast_to((NPOS, F, DO))
                nc.vector.tensor_mul(
                    dst_ap, s[:].rearrange("p (f o) -> p f o", o=DO), bc
                )

            # ---------------- stage B: out0 ----------------
            o0p = acc.tile([NPOS, F * DO], F32, tag="mm")
            for t in range(NCHUNK):
                nc.tensor.matmul(
                    o0p[:],
                    p_ct[:, t * NPOS:(t + 1) * NPOS],
                    w_r[:, t * F * DO:(t + 1) * F * DO],
                    start=(t == 0),
                    stop=(t == NCHUNK - 1),
                )
            out0_pad = work.tile([NPOS, F * 32], F32, tag="out0_pad")
            nc.vector.memset(out0_pad[:], 0.0)
            squash(
                o0p[:],
                out0_pad[:].rearrange("p (f s) -> p f s", s=32)[:, :, 0:DO],
                1.0 / F,
                "sq1",
            )

            # transposes -> tpq [128=(4l,32slot), 8Q*72], block-diag bd
            tpq = work.tile([128, NQRT * NPOS], F16, tag="tpq")
            for Q in range(NQRT):
                tp = tps.tile([128, 128], F32, tag="tp")
                nc.tensor.transpose(
                    tp[:, 0:NPOS], out0_pad[:, Q * 128:(Q + 1) * 128], eye72[:]
                )
                nc.scalar.copy(tpq[:, Q * NPOS:(Q + 1) * NPOS], tp[:, 0:NPOS])
            bd = work.tile([128, NQRT * QBD], F16, tag="bd")
            nc.vector.memset(bd[:], 0.0)
            for Q in range(NQRT):
                for l in range(4):
                    nc.vector.tensor_copy(
                        bd[32 * l:32 * l + 32,
                           Q * QBD + l * NPOS:Q * QBD + (l + 1) * NPOS],
                        tpq[32 * l:32 * l + 32, Q * NPOS:(Q + 1) * NPOS],
                    )

            # ---------------- V + VP + agr + exp ----------------
            # e[c; (Q,cb,l,pos)] = exp(agr), SBUF fp16
            e = work.tile([128, NQRT * CBLK * QBD], F16, tag="e")
            for Q in range(NQRT):
                VP = ring.tile([128, NCHUNK * QBD], F16, tag="VP")
                for t in range(NCHUNK):
                    vh = vps.tile([128, QBD], F32, tag="vh")
                    nc.tensor.matmul(
                        vh[:],
                        w8[:, (Q * NCHUNK + t) * 128:(Q * NCHUNK + t + 1) * 128],
                        bd[:, Q * QBD:(Q + 1) * QBD],
                        start=True,
                        stop=True,
                    )
                    # VP = V * P (P broadcast over the 4 f's of the quartet)
                    pb = (
                        p_ct[:, t * NPOS:(t + 1) * NPOS]
                        .unsqueeze(1)
                        .broadcast_to((128, 4, NPOS))
                    )
                    if t % 2 == 0:
                        nc.vector.tensor_mul(
                            VP[:, t * QBD:(t + 1) * QBD]
                            .rearrange("p (j n) -> p j n", n=NPOS),
                            vh[:].rearrange("p (j n) -> p j n", n=NPOS),
                            pb,
                        )
                    else:
                        vs = ring.tile([128, QBD], F16, tag="vs")
                        nc.scalar.copy(vs[:], vh[:])
                        nc.vector.tensor_mul(
                            VP[:, t * QBD:(t + 1) * QBD]
                            .rearrange("p (j n) -> p j n", n=NPOS),
                            vs[:].rearrange("p (j n) -> p j n", n=NPOS),
                            pb,
                        )
                # agr = sum_i VP (identity-matmul accumulation), then exp
                for cb in range(CBLK):
                    agp = acc.tile([128, QBD], F32, tag="mm")
                    for i in range(DI):
                        t = i * CBLK + cb
                        nc.tensor.matmul(
                            agp[:],
                            eye128h[:],
                            VP[:, t * QBD:(t + 1) * QBD],
                            start=(i == 0),
                            stop=(i == DI - 1),
                        )
                    nc.scalar.activation(
                        e[:, (Q * CBLK + cb) * QBD:(Q * CBLK + cb + 1) * QBD],
                        agp[:],
                        AF.Exp,
                    )

            # ---------------- softmax normalizer + pp ----------------
            Zp = work.tile([128, NQRT * CBLK * NPOS], F32, tag="Zp")
            for Q in range(NQRT):
                nc.vector.reduce_sum(
                    Zp[:, Q * CBLK * NPOS:(Q + 1) * CBLK * NPOS]
                    .rearrange("p (cb n) -> p cb n", cb=CBLK),
                    e[:, Q * CBLK * QBD:(Q + 1) * CBLK * QBD]
                    .rearrange("p (cb l n) -> p cb n l", cb=CBLK, l=4),
                    axis=AX.X,
                )
            CN = CBLK * NPOS
            Z4 = work.tile([128, 4 * CN], F32, tag="Z4")
            for m in range(4):
                nc.vector.tensor_add(
                    Z4[:, m * CN:(m + 1) * CN],
                    Zp[:, 2 * m * CN:(2 * m + 1) * CN],
                    Zp[:, (2 * m + 1) * CN:(2 * m + 2) * CN],
                )
            Z2 = work.tile([128, 2 * CN], F32, tag="Z2")
            for m in range(2):
                nc.vector.tensor_add(
                    Z2[:, m * CN:(m + 1) * CN],
                    Z4[:, 2 * m * CN:(2 * m + 1) * CN],
                    Z4[:, (2 * m + 1) * CN:(2 * m + 2) * CN],
                )
            Zf = work.tile([128, CN], F32, tag="Zf")
            nc.vector.tensor_add(Zf[:], Z2[:, 0:CN], Z2[:, CN:2 * CN])
            Zr = work.tile([128, CN], F32, tag="Zr")
            nc.vector.reciprocal(Zr[:], Zf[:])
            pp = work.tile([128, NCHUNK * NPOS], F16, tag="pp")
            nc.vector.tensor_mul(
                pp[:].rearrange("p (i cb n) -> p i cb n", i=DI, cb=CBLK),
                p_ct[:].rearrange("p (i cb n) -> p i cb n", i=DI, cb=CBLK),
                Zr[:].rearrange("p (cb n) -> p cb n", cb=CBLK)
                .unsqueeze(1)
                .broadcast_to((128, DI, CBLK, NPOS)),
            )

            # ---------------- S + centroids ----------------
            cen_sb = work.tile([128, 8 * NPOS], F32, tag="cen_sb")
            for g in range(8):
                cg = cenp.tile([128, NPOS], F32, tag="cen")
                Ss = []
                for j in range(4):
                    S = ring.tile([128, NCHUNK * NPOS], F16, tag=f"S{j}")
                    eb = (
                        e[:, g * CBLK * QBD:(g + 1) * CBLK * QBD]
                        .rearrange("p (cb l n) -> p cb l n", cb=CBLK, l=4)
                        [:, :, j, :]
                        .unsqueeze(1)
                        .broadcast_to((128, DI, CBLK, NPOS))
                    )
                    nc.vector.tensor_mul(
                        S[:].rearrange("p (i cb n) -> p i cb n", i=DI, cb=CBLK),
                        pp[:].rearrange("p (i cb n) -> p i cb n", i=DI, cb=CBLK),
                        eb,
                    )
                    Ss.append(S)
                for t in range(NCHUNK):
                    for j in range(4):
                        f = 4 * g + j
                        nc.tensor.matmul(
                            cg[32 * j:32 * j + DO, :],
                            w_r[:, t * F * DO + f * DO:t * F * DO + (f + 1) * DO],
                            Ss[j][:, t * NPOS:(t + 1) * NPOS],
                            start=(t == 0),
                            stop=(t == NCHUNK - 1),
                            tile_position=(0, 32 * j),
                        )
                nc.scalar.copy(cen_sb[:, g * NPOS:(g + 1) * NPOS], cg[:])

            # ---------------- squash2 + output ----------------
            opre = work.tile([NPOS, 8 * 128], F32, tag="opre")
            for g in range(8):
                tp2 = tps.tile([128, 128], F32, tag="tp")
                nc.tensor.transpose(
                    tp2[0:NPOS, :], cen_sb[:, g * NPOS:(g + 1) * NPOS], eye128f[:]
                )
                nc.scalar.copy(opre[:, g * 128:(g + 1) * 128], tp2[0:NPOS, :])
            gat = work.tile([NPOS, F * DO], F32, tag="gat")
            nc.vector.tensor_copy(
                gat[:].rearrange("p (g j o) -> p g j o", g=8, j=4),
                opre[:].rearrange("p (g j s) -> p g j s", g=8, j=4)[:, :, :, 0:DO],
            )
            y_sb = work.tile([NPOS, F * DO], F32, tag="y_sb")
            squash(gat[:], y_sb[:].rearrange("p (f o) -> p f o", o=DO), 1.0, "sq2")
            nc.sync.dma_start(y_d[:], y_sb[:])

    nc.compile()
    return nc


def _get_program():
    if "nc" not in _CACHE:
        _CACHE["nc"] = _build()
    return _CACHE["nc"]


def _in_maps(x, Wm):
    w_r, w8 = _host_weights(Wm)
    eye72 = np.eye(NPOS, dtype=np.float32)
    eye128h = np.eye(128, dtype=np.float16)
    eye128f = np.eye(128, dtype=np.float32)
    return [{
        "p_ct": _host_patches(x, k),
        "w_r": w_r,
        "w8": w8,
        "eye72": eye72,
        "eye128h": eye128h,
        "eye128f": eye128f,
    } for k in range(8)]


def kernel(x, W):
    from concourse.bass_utils import run_bass_kernel_spmd

    x = np.asarray(x, np.float32)
    Wm = np.asarray(W, np.float32)[0, 0, 0]
    nc = _get_program()
    res = run_bass_kernel_spmd(nc, _in_maps(x, Wm), list(range(8)))
    Ho, Wo = H - KH + 1, WD - KW + 1
    y = np.empty((B, Ho, Wo, F, DO), np.float32)
    for k in range(8):
        b, hh = divmod(k, 2)
        y[b, 6 * hh:6 * hh + 6] = res.results[k]["y"].reshape(6, Wo, F, DO)
    return y


# revision 8
# speedup vs baseline: 1.6291x; 1.1282x over previous
"""Trainium2 Bass kernel for the ConvolutionalCapsule module.

Sharding: data-parallel over (batch, H-half): core k handles b = k//2,
output rows h in [6*(k%2), 6*(k%2)+6), i.e. 72 spatial positions per core.
Weights are replicated. All FLOPs run on-device; the host only does
layout/gather of inputs (patch extraction + weight transposes).

Device algorithm per core (pos = 72 positions), chunk layout keeps
(c-block-of-128) on SBUF partitions, chunks indexed t = i*3 + cb:
  iter0:  out0 = squash(mean_c preds) via one K=(c,i)=2304 matmul.
  iter1:  V[f,c,i] = sum_o W[f,c,o,i]*out0[f,o] via W-stationary matmuls
            [(4f,32)=128, c-chunk] streaming a block-diagonal out0
            (8 quartet tiles [128, 4f*72], built once, 32-aligned blocks).
          VP = V (.) P  (PSUM exit fused / split scalar+vector)
          agr[c, (f,pos)] = sum_i VP  (identity-matmul PSUM accumulation)
          e = exp(agr)  (scalar engine, fused with PSUM exit)
          Z = sum_f e ; pp = P * (1/Z)
          S_f = e_f (.) pp ; centroids via col-tiled w_r matmuls
          out = squash(centroids)
"""
import numpy as np

KH = KW = 3
B, H, WD, FIN, DIN = 4, 14, 14, 32, 8
F, C, DO, DI = 32, 288, 16, 8
NPOS = 72
CBLK = 3
NCHUNK = DI * CBLK  # 24
NQRT = 8            # f-quartets of 4
EPS = 1e-7

_CACHE: dict = {}


def _chunk_rows(t):
    i, cb = divmod(t, CBLK)
    c0 = cb * 128
    return i, c0, min(128, C - c0)


def _host_weights(Wm):
    """Wm: [F, C, DO, DI] float32 -> device weight layouts (fp16)."""
    w_r = np.zeros((NCHUNK, 128, F * DO), np.float16)
    for t in range(NCHUNK):
        i, c0, n = _chunk_rows(t)
        w_r[t, :n, :] = (
            Wm[:, c0:c0 + n, :, i].transpose(1, 0, 2).reshape(n, F * DO)
        )
    w_r = w_r.transpose(1, 0, 2).reshape(128, NCHUNK * F * DO).copy()
    # w8[(l,o-slot32), (Q,t,c)]: stationary V-matmul weights
    w8 = np.zeros((NQRT, NCHUNK, 128, 128), np.float16)
    for Q in range(NQRT):
        for t in range(NCHUNK):
            i, c0, n = _chunk_rows(t)
            for l in range(4):
                f = 4 * Q + l
                w8[Q, t, 32 * l:32 * l + DO, :n] = Wm[f, c0:c0 + n, :, i].T
    w8 = w8.transpose(2, 0, 1, 3).reshape(128, NQRT * NCHUNK * 128).copy()
    return w_r, w8


def _host_patches(x, k):
    """Patch tensor for core k in (i, c-block) chunk layout: [128, 24*72]."""
    b, hh = divmod(k, 2)
    h0 = 6 * hh
    P = np.empty((6, 12, KH, KW, FIN, DIN), np.float32)
    for kh in range(KH):
        for kw in range(KW):
            for h in range(6):
                P[h, :, kh, kw] = x[b, h0 + h + kh, kw:kw + 12]
    P = P.reshape(NPOS, C, DIN)
    p_ct = np.zeros((NCHUNK, 128, NPOS), np.float16)
    for t in range(NCHUNK):
        i, c0, n = _chunk_rows(t)
        p_ct[t, :n, :] = P[:, c0:c0 + n, i].T
    return p_ct.transpose(1, 0, 2).reshape(128, NCHUNK * NPOS).copy()


def _build():
    import concourse.bass as bass
    import concourse.bacc as bacc
    import concourse.mybir as mybir
    import concourse.tile as tile

    F16, F32 = mybir.dt.float16, mybir.dt.float32
    AX = mybir.AxisListType
    AF = mybir.ActivationFunctionType

    nc = bacc.Bacc(None, target_bir_lowering=False, debug=False)

    p_ct_d = nc.dram_tensor("p_ct", [128, NCHUNK * NPOS], F16, kind="ExternalInput")
    w_r_d = nc.dram_tensor("w_r", [128, NCHUNK * F * DO], F16, kind="ExternalInput")
    w8_d = nc.dram_tensor("w8", [128, NQRT * NCHUNK * 128], F16, kind="ExternalInput")
    eye72_d = nc.dram_tensor("eye72", [NPOS, NPOS], F32, kind="ExternalInput")
    eye128h_d = nc.dram_tensor("eye128h", [128, 128], F16, kind="ExternalInput")
    eye128f_d = nc.dram_tensor("eye128f", [128, 128], F32, kind="ExternalInput")
    y_d = nc.dram_tensor("y", [NPOS, F * DO], F32, kind="ExternalOutput")

    QBD = 4 * NPOS            # 288 = 4f * 72pos, one quartet's free extent

    with tile.TileContext(nc) as tc:
        with (
            tc.tile_pool(name="const", bufs=1) as const,
            tc.tile_pool(name="work", bufs=1) as work,
            tc.tile_pool(name="ring", bufs=2) as ring,
            tc.tile_pool(name="vps", bufs=2, space=bass.MemorySpace.PSUM) as vps,
            tc.tile_pool(name="tps", bufs=2, space=bass.MemorySpace.PSUM) as tps,
            tc.tile_pool(name="acc", bufs=2, space=bass.MemorySpace.PSUM) as acc,
            tc.tile_pool(name="cenp", bufs=2, space=bass.MemorySpace.PSUM) as cenp,
        ):
            # ---------------- loads (chunked so compute starts early) ----------------
            p_ct = const.tile([128, NCHUNK * NPOS], F16, tag="p_ct")
            nc.sync.dma_start(p_ct[:], p_ct_d[:])
            w_r = const.tile([128, NCHUNK * F * DO], F16, tag="w_r")
            WRS = NCHUNK * F * DO // 6
            for s in range(6):
                nc.sync.dma_start(
                    w_r[:, s * WRS:(s + 1) * WRS], w_r_d[:, s * WRS:(s + 1) * WRS]
                )
            eye72 = const.tile([NPOS, NPOS], F32, tag="eye72")
            nc.sync.dma_start(eye72[:], eye72_d[:])
            eye128h = const.tile([128, 128], F16, tag="eye128h")
            nc.sync.dma_start(eye128h[:], eye128h_d[:])
            eye128f = const.tile([128, 128], F32, tag="eye128f")
            nc.sync.dma_start(eye128f[:], eye128f_d[:])
            w8 = const.tile([128, NQRT * NCHUNK * 128], F16, tag="w8")
            W8S = NCHUNK * 128
            for s in range(NQRT):
                nc.sync.dma_start(
                    w8[:, s * W8S:(s + 1) * W8S], w8_d[:, s * W8S:(s + 1) * W8S]
                )

            def squash(src_ap, dst_ap, pre_scale, tag):
                """dst = squash(src * pre_scale) ; src free = (f,o)=512."""
                s = work.tile([NPOS, F * DO], F32, tag=f"{tag}_s")
                nc.scalar.activation(s[:], src_ap, AF.Copy, scale=pre_scale)
                sq = work.tile([NPOS, F * DO], F32, tag=f"{tag}_sq")
                nc.scalar.activation(sq[:], s[:], AF.Square)
                sn = work.tile([NPOS, F], F32, tag=f"{tag}_sn")
                nc.vector.reduce_sum(
                    sn[:], sq[:].rearrange("p (f o) -> p f o", o=DO), axis=AX.X
                )
                t1 = work.tile([NPOS, F], F32, tag=f"{tag}_t1")
                nc.vector.tensor_scalar_add(t1[:], sn[:], 1.0)
                r1 = work.tile([NPOS, F], F32, tag=f"{tag}_r1")
                nc.vector.reciprocal(r1[:], t1[:])
                se = work.tile([NPOS, F], F32, tag=f"{tag}_se")
                nc.vector.tensor_scalar_add(se[:], sn[:], EPS)
                r2 = work.tile([NPOS, F], F32, tag=f"{tag}_r2")
                nc.scalar.activation(r2[:], se[:], AF.Sqrt)
                r3 = work.tile([NPOS, F], F32, tag=f"{tag}_r3")
                nc.vector.reciprocal(r3[:], r2[:])
                sc = work.tile([NPOS, F], F32, tag=f"{tag}_sc")
                nc.vector.tensor_mul(sc[:], sn[:], r1[:])
                sc2 = work.tile([NPOS, F], F32, tag=f"{tag}_sc2")
                nc.vector.tensor_mul(sc2[:], sc[:], r3[:])
                bc = sc2[:].unsqueeze(2).broadcast_to((NPOS, F, DO))
                nc.vector.tensor_mul(
                    dst_ap, s[:].rearrange("p (f o) -> p f o", o=DO), bc
                )

            # ---------------- stage B: out0 ----------------
            o0p = acc.tile([NPOS, F * DO], F32, tag="mm")
            for t in range(NCHUNK):
                nc.tensor.matmul(
                    o0p[:],
                    p_ct[:, t * NPOS:(t + 1) * NPOS],
                    w_r[:, t * F * DO:(t + 1) * F * DO],
                    start=(t == 0),
                    stop=(t == NCHUNK - 1),
                )
            out0_pad = work.tile([NPOS, F * 32], F32, tag="out0_pad")
            nc.vector.memset(out0_pad[:], 0.0)
            squash(
                o0p[:],
                out0_pad[:].rearrange("p (f s) -> p f s", s=32)[:, :, 0:DO],
                1.0 / F,
                "sq1",
            )

            # transposes -> tpq [128=(4l,32slot), 8Q*72], block-diag bd
            tpq = work.tile([128, NQRT * NPOS], F16, tag="tpq")
            for Q in range(NQRT):
                tp = tps.tile([128, 128], F32, tag="tp")
                nc.tensor.transpose(
                    tp[:, 0:NPOS], out0_pad[:, Q * 128:(Q + 1) * 128], eye72[:]
                )
                nc.scalar.copy(tpq[:, Q * NPOS:(Q + 1) * NPOS], tp[:, 0:NPOS])
            bd = work.tile([128, NQRT * QBD], F16, tag="bd")
            nc.vector.memset(bd[:], 0.0)
            for Q in range(NQRT):
                for l in range(4):
                    nc.vector.tensor_copy(
                        bd[32 * l:32 * l + 32,
                           Q * QBD + l * NPOS:Q * QBD + (l + 1) * NPOS],
                        tpq[32 * l:32 * l + 32, Q * NPOS:(Q + 1) * NPOS],
                    )

            # ---------------- V + VP + agr + exp (software-pipelined) ----------------
            # e[c; (Q,cb,l,pos)] = exp(agr), SBUF fp16
            e = work.tile([128, NQRT * CBLK * QBD], F16, tag="e")

            def ired_phase(Q, VP):
                # agr = sum_i VP (identity-matmul accumulation), then exp
                for cb in range(CBLK):
                    agp = acc.tile([128, QBD], F32, tag="mm")
                    for i in range(DI):
                        t = i * CBLK + cb
                        nc.tensor.matmul(
                            agp[:],
                            eye128h[:],
                            VP[:, t * QBD:(t + 1) * QBD],
                            start=(i == 0),
                            stop=(i == DI - 1),
                        )
                    nc.scalar.activation(
                        e[:, (Q * CBLK + cb) * QBD:(Q * CBLK + cb + 1) * QBD],
                        agp[:],
                        AF.Exp,
                    )

            VPs = []
            for Q in range(NQRT):
                VP = ring.tile([128, NCHUNK * QBD], F16, tag="VP")
                for t in range(NCHUNK):
                    vh = vps.tile([128, QBD], F32, tag="vh")
                    nc.tensor.matmul(
                        vh[:],
                        w8[:, (Q * NCHUNK + t) * 128:(Q * NCHUNK + t + 1) * 128],
                        bd[:, Q * QBD:(Q + 1) * QBD],
                        start=True,
                        stop=True,
                    )
                    # VP = V * P (P broadcast over the 4 f's of the quartet)
                    pb = (
                        p_ct[:, t * NPOS:(t + 1) * NPOS]
                        .unsqueeze(1)
                        .broadcast_to((128, 4, NPOS))
                    )
                    if t % 3 == 2:
                        nc.vector.tensor_mul(
                            VP[:, t * QBD:(t + 1) * QBD]
                            .rearrange("p (j n) -> p j n", n=NPOS),
                            vh[:].rearrange("p (j n) -> p j n", n=NPOS),
                            pb,
                        )
                    else:
                        vs = ring.tile([128, QBD], F16, tag="vs")
                        nc.scalar.copy(vs[:], vh[:])
                        nc.vector.tensor_mul(
                            VP[:, t * QBD:(t + 1) * QBD]
                            .rearrange("p (j n) -> p j n", n=NPOS),
                            vs[:].rearrange("p (j n) -> p j n", n=NPOS),
                            pb,
                        )
                VPs.append(VP)
                if Q > 0:
                    ired_phase(Q - 1, VPs[Q - 1])
            ired_phase(NQRT - 1, VPs[NQRT - 1])

            # ---------------- softmax normalizer + pp ----------------
            CN = CBLK * NPOS
            Zp = work.tile([128, NQRT * CN], F32, tag="Zp")
            for Q in range(NQRT):
                eq = e[:, Q * CBLK * QBD:(Q + 1) * CBLK * QBD].rearrange(
                    "p (cb l n) -> p cb l n", cb=CBLK, l=4
                )
                za = work.tile([128, CN], F32, tag="za")
                nc.vector.tensor_add(
                    za[:].rearrange("p (cb n) -> p cb n", cb=CBLK),
                    eq[:, :, 0, :], eq[:, :, 1, :],
                )
                zb = work.tile([128, CN], F32, tag="zb")
                nc.vector.tensor_add(
                    zb[:].rearrange("p (cb n) -> p cb n", cb=CBLK),
                    eq[:, :, 2, :], eq[:, :, 3, :],
                )
                nc.vector.tensor_add(
                    Zp[:, Q * CN:(Q + 1) * CN], za[:], zb[:]
                )
            Z4 = work.tile([128, 4 * CN], F32, tag="Z4")
            for m in range(4):
                nc.vector.tensor_add(
                    Z4[:, m * CN:(m + 1) * CN],
                    Zp[:, 2 * m * CN:(2 * m + 1) * CN],
                    Zp[:, (2 * m + 1) * CN:(2 * m + 2) * CN],
                )
            Z2 = work.tile([128, 2 * CN], F32, tag="Z2")
            for m in range(2):
                nc.vector.tensor_add(
                    Z2[:, m * CN:(m + 1) * CN],
                    Z4[:, 2 * m * CN:(2 * m + 1) * CN],
                    Z4[:, (2 * m + 1) * CN:(2 * m + 2) * CN],
                )
            Zf = work.tile([128, CN], F32, tag="Zf")
            nc.vector.tensor_add(Zf[:], Z2[:, 0:CN], Z2[:, CN:2 * CN])
            Zr = work.tile([128, CN], F32, tag="Zr")
            nc.vector.reciprocal(Zr[:], Zf[:])
            pp = work.tile([128, NCHUNK * NPOS], F16, tag="pp")
            nc.vector.tensor_mul(
                pp[:].rearrange("p (i cb n) -> p i cb n", i=DI, cb=CBLK),
                p_ct[:].rearrange("p (i cb n) -> p i cb n", i=DI, cb=CBLK),
                Zr[:].rearrange("p (cb n) -> p cb n", cb=CBLK)
                .unsqueeze(1)
                .broadcast_to((128, DI, CBLK, NPOS)),
            )

            # ---------------- S + centroids ----------------
            cen_sb = work.tile([128, 8 * NPOS], F32, tag="cen_sb")
            for g in range(8):
                cg = cenp.tile([128, NPOS], F32, tag="cen")
                Ss = []
                for j in range(4):
                    S = ring.tile([128, NCHUNK * NPOS], F16, tag=f"S{j}")
                    eb = (
                        e[:, g * CBLK * QBD:(g + 1) * CBLK * QBD]
                        .rearrange("p (cb l n) -> p cb l n", cb=CBLK, l=4)
                        [:, :, j, :]
                        .unsqueeze(1)
                        .broadcast_to((128, DI, CBLK, NPOS))
                    )
                    nc.vector.tensor_mul(
                        S[:].rearrange("p (i cb n) -> p i cb n", i=DI, cb=CBLK),
                        pp[:].rearrange("p (i cb n) -> p i cb n", i=DI, cb=CBLK),
                        eb,
                    )
                    Ss.append(S)
                for t in range(NCHUNK):
                    for j in range(4):
                        f = 4 * g + j
                        nc.tensor.matmul(
                            cg[32 * j:32 * j + DO, :],
                            w_r[:, t * F * DO + f * DO:t * F * DO + (f + 1) * DO],
                            Ss[j][:, t * NPOS:(t + 1) * NPOS],
                            start=(t == 0),
                            stop=(t == NCHUNK - 1),
                            tile_position=(0, 32 * j),
                        )
                nc.scalar.copy(cen_sb[:, g * NPOS:(g + 1) * NPOS], cg[:])

            # ---------------- squash2 + output ----------------
            opre = work.tile([NPOS, 8 * 128], F32, tag="opre")
            for g in range(8):
                tp2 = tps.tile([128, 128], F32, tag="tp")
                nc.tensor.transpose(
                    tp2[0:NPOS, :], cen_sb[:, g * NPOS:(g + 1) * NPOS], eye128f[:]
                )
                nc.scalar.copy(opre[:, g * 128:(g + 1) * 128], tp2[0:NPOS, :])
            gat = work.tile([NPOS, F * DO], F32, tag="gat")
            nc.vector.tensor_copy(
                gat[:].rearrange("p (g j o) -> p g j o", g=8, j=4),
                opre[:].rearrange("p (g j s) -> p g j s", g=8, j=4)[:, :, :, 0:DO],
            )
            y_sb = work.tile([NPOS, F * DO], F32, tag="y_sb")
            squash(gat[:], y_sb[:].rearrange("p (f o) -> p f o", o=DO), 1.0, "sq2")
            nc.sync.dma_start(y_d[:], y_sb[:])

    nc.compile()
    return nc


def _get_program():
    if "nc" not in _CACHE:
        _CACHE["nc"] = _build()
    return _CACHE["nc"]


def _in_maps(x, Wm):
    w_r, w8 = _host_weights(Wm)
    eye72 = np.eye(NPOS, dtype=np.float32)
    eye128h = np.eye(128, dtype=np.float16)
    eye128f = np.eye(128, dtype=np.float32)
    return [{
        "p_ct": _host_patches(x, k),
        "w_r": w_r,
        "w8": w8,
        "eye72": eye72,
        "eye128h": eye128h,
        "eye128f": eye128f,
    } for k in range(8)]


def kernel(x, W):
    from concourse.bass_utils import run_bass_kernel_spmd

    x = np.asarray(x, np.float32)
    Wm = np.asarray(W, np.float32)[0, 0, 0]
    nc = _get_program()
    res = run_bass_kernel_spmd(nc, _in_maps(x, Wm), list(range(8)))
    Ho, Wo = H - KH + 1, WD - KW + 1
    y = np.empty((B, Ho, Wo, F, DO), np.float32)
    for k in range(8):
        b, hh = divmod(k, 2)
        y[b, 6 * hh:6 * hh + 6] = res.results[k]["y"].reshape(6, Wo, F, DO)
    return y


# revision 14
# speedup vs baseline: 1.8763x; 1.1518x over previous
"""Trainium2 Bass kernel for the ConvolutionalCapsule module.

Sharding: data-parallel over (batch, H-half): core k handles b = k//2,
output rows h in [6*(k%2), 6*(k%2)+6), i.e. 72 spatial positions per core.
Weights are replicated. All FLOPs run on-device; the host only does
layout/gather of inputs (patch extraction + weight transposes).

Device algorithm per core (pos = 72 positions), chunk layout keeps
(c-block-of-128) on SBUF partitions, chunks indexed t = i*3 + cb:
  iter0:  out0 = squash(mean_c preds) via one K=(c,i)=2304 matmul.
  iter1:  V[f,c,i] = sum_o W[f,c,o,i]*out0[f,o] via W-stationary matmuls
            [(4f,32)=128, c-chunk] streaming a block-diagonal out0
            (8 quartet tiles [128, 4f*72], built once, 32-aligned blocks).
          VP = V (.) P  (PSUM exit fused / split scalar+vector)
          agr[c, (f,pos)] = sum_i VP  (identity-matmul PSUM accumulation)
          e = exp(agr)  (scalar engine, fused with PSUM exit)
          Z = sum_f e ; pp = P * (1/Z)
          S_f = e_f (.) pp ; centroids via col-tiled w_r matmuls
          out = squash(centroids)
"""
import numpy as np

KH = KW = 3
B, H, WD, FIN, DIN = 4, 14, 14, 32, 8
F, C, DO, DI = 32, 288, 16, 8
NPOS = 72
CBLK = 3
NCHUNK = DI * CBLK  # 24
NQRT = 8            # f-quartets of 4
EPS = 1e-7

_CACHE: dict = {}


def _chunk_rows(t):
    i, cb = divmod(t, CBLK)
    c0 = cb * 128
    return i, c0, min(128, C - c0)


def _host_weights(Wm):
    """Wm: [F, C, DO, DI] float32 -> device weight layouts (fp16)."""
    w_r = np.zeros((NCHUNK, 128, F * DO), np.float16)
    for t in range(NCHUNK):
        i, c0, n = _chunk_rows(t)
        w_r[t, :n, :] = (
            Wm[:, c0:c0 + n, :, i].transpose(1, 0, 2).reshape(n, F * DO)
        )
    w_r = w_r.transpose(1, 0, 2).reshape(128, NCHUNK * F * DO).copy()
    # w8[(l,o-slot32), (Q,t,c)]: stationary V-matmul weights
    w8 = np.zeros((NQRT, NCHUNK, 128, 128), np.float16)
    for Q in range(NQRT):
        for t in range(NCHUNK):
            i, c0, n = _chunk_rows(t)
            for l in range(4):
                f = 4 * Q + l
                w8[Q, t, 32 * l:32 * l + DO, :n] = Wm[f, c0:c0 + n, :, i].T
    w8 = w8.transpose(2, 0, 1, 3).reshape(128, NQRT * NCHUNK * 128).copy()
    return w_r, w8


def _host_patches(x, k):
    """Patch tensor for core k in (i, c-block) chunk layout: [128, 24*72]."""
    b, hh = divmod(k, 2)
    h0 = 6 * hh
    P = np.empty((6, 12, KH, KW, FIN, DIN), np.float32)
    for kh in range(KH):
        for kw in range(KW):
            for h in range(6):
                P[h, :, kh, kw] = x[b, h0 + h + kh, kw:kw + 12]
    P = P.reshape(NPOS, C, DIN)
    p_ct = np.zeros((NCHUNK, 128, NPOS), np.float16)
    for t in range(NCHUNK):
        i, c0, n = _chunk_rows(t)
        p_ct[t, :n, :] = P[:, c0:c0 + n, i].T
    return p_ct.transpose(1, 0, 2).reshape(128, NCHUNK * NPOS).copy()


def _build():
    import concourse.bass as bass
    import concourse.bacc as bacc
    import concourse.mybir as mybir
    import concourse.tile as tile

    F16, F32 = mybir.dt.float16, mybir.dt.float32
    AX = mybir.AxisListType
    AF = mybir.ActivationFunctionType

    nc = bacc.Bacc(None, target_bir_lowering=False, debug=False)

    p_ct_d = nc.dram_tensor("p_ct", [128, NCHUNK * NPOS], F16, kind="ExternalInput")
    w_r_d = nc.dram_tensor("w_r", [128, NCHUNK * F * DO], F16, kind="ExternalInput")
    w8_d = nc.dram_tensor("w8", [128, NQRT * NCHUNK * 128], F16, kind="ExternalInput")
    eye72_d = nc.dram_tensor("eye72", [NPOS, NPOS], F32, kind="ExternalInput")
    eye128h_d = nc.dram_tensor("eye128h", [128, 128], F16, kind="ExternalInput")
    eye128f_d = nc.dram_tensor("eye128f", [128, 128], F32, kind="ExternalInput")
    y_d = nc.dram_tensor("y", [NPOS, F * DO], F32, kind="ExternalOutput")

    QBD = 4 * NPOS            # 288 = 4f * 72pos, one quartet's free extent

    with tile.TileContext(nc) as tc:
        with (
            tc.tile_pool(name="const", bufs=1) as const,
            tc.tile_pool(name="work", bufs=1) as work,
            tc.tile_pool(name="ring", bufs=2) as ring,
            tc.tile_pool(name="vsr", bufs=4) as vsr,
            tc.tile_pool(name="vps", bufs=2, space=bass.MemorySpace.PSUM) as vps,
            tc.tile_pool(name="tps", bufs=1, space=bass.MemorySpace.PSUM) as tps,
            tc.tile_pool(name="acc", bufs=2, space=bass.MemorySpace.PSUM) as acc,
            tc.tile_pool(name="cenp", bufs=1, space=bass.MemorySpace.PSUM) as cenp,
        ):
            # ---------------- loads (chunked so compute starts early) ----------------
            p_ct = const.tile([128, NCHUNK * NPOS], F16, tag="p_ct")
            w_r = const.tile([128, NCHUNK * F * DO], F16, tag="w_r")
            PCS = NCHUNK * NPOS // 4
            WRS = NCHUNK * F * DO // 6
            nc.sync.dma_start(p_ct[:, 0:PCS], p_ct_d[:, 0:PCS])
            nc.sync.dma_start(w_r[:, 0:WRS], w_r_d[:, 0:WRS])
            for s in range(1, 4):
                nc.sync.dma_start(
                    p_ct[:, s * PCS:(s + 1) * PCS], p_ct_d[:, s * PCS:(s + 1) * PCS]
                )
            for s in range(1, 6):
                nc.sync.dma_start(
                    w_r[:, s * WRS:(s + 1) * WRS], w_r_d[:, s * WRS:(s + 1) * WRS]
                )
            eye72 = const.tile([NPOS, NPOS], F32, tag="eye72")
            nc.sync.dma_start(eye72[:], eye72_d[:])
            eye128h = const.tile([128, 128], F16, tag="eye128h")
            nc.sync.dma_start(eye128h[:], eye128h_d[:])
            eye128f = const.tile([128, 128], F32, tag="eye128f")
            nc.sync.dma_start(eye128f[:], eye128f_d[:])
            w8 = const.tile([128, NQRT * NCHUNK * 128], F16, tag="w8")
            W8S = NCHUNK * 128
            for s in range(NQRT):
                nc.sync.dma_start(
                    w8[:, s * W8S:(s + 1) * W8S], w8_d[:, s * W8S:(s + 1) * W8S]
                )

            def squash(src_ap, dst_ap, pre_scale, tag):
                """dst = squash(src * pre_scale) ; src free = (f,o)=512."""
                s = work.tile([NPOS, F * DO], F32, tag=f"{tag}_s")
                nc.scalar.activation(s[:], src_ap, AF.Copy, scale=pre_scale)
                sq = work.tile([NPOS, F * DO], F32, tag=f"{tag}_sq")
                nc.scalar.activation(sq[:], s[:], AF.Square)
                sn = work.tile([NPOS, F], F32, tag=f"{tag}_sn")
                nc.vector.reduce_sum(
                    sn[:], sq[:].rearrange("p (f o) -> p f o", o=DO), axis=AX.X
                )
                t1 = work.tile([NPOS, F], F32, tag=f"{tag}_t1")
                nc.vector.tensor_scalar_add(t1[:], sn[:], 1.0)
                r1 = work.tile([NPOS, F], F32, tag=f"{tag}_r1")
                nc.vector.reciprocal(r1[:], t1[:])
                se = work.tile([NPOS, F], F32, tag=f"{tag}_se")
                nc.vector.tensor_scalar_add(se[:], sn[:], EPS)
                r2 = work.tile([NPOS, F], F32, tag=f"{tag}_r2")
                nc.scalar.activation(r2[:], se[:], AF.Sqrt)
                r3 = work.tile([NPOS, F], F32, tag=f"{tag}_r3")
                nc.vector.reciprocal(r3[:], r2[:])
                sc = work.tile([NPOS, F], F32, tag=f"{tag}_sc")
                nc.vector.tensor_mul(sc[:], sn[:], r1[:])
                sc2 = work.tile([NPOS, F], F32, tag=f"{tag}_sc2")
                nc.vector.tensor_mul(sc2[:], sc[:], r3[:])
                bc = sc2[:].unsqueeze(2).broadcast_to((NPOS, F, DO))
                nc.vector.tensor_mul(
                    dst_ap, s[:].rearrange("p (f o) -> p f o", o=DO), bc
                )

            # ---------------- stage B: out0 ----------------
            o0p = acc.tile([NPOS, F * DO], F32, tag="mm")
            for t in range(NCHUNK):
                nc.tensor.matmul(
                    o0p[:],
                    p_ct[:, t * NPOS:(t + 1) * NPOS],
                    w_r[:, t * F * DO:(t + 1) * F * DO],
                    start=(t == 0),
                    stop=(t == NCHUNK - 1),
                )
            out0_pad = work.tile([NPOS, F * 32], F32, tag="out0_pad")
            nc.vector.memset(out0_pad[:], 0.0)
            squash(
                o0p[:],
                out0_pad[:].rearrange("p (f s) -> p f s", s=32)[:, :, 0:DO],
                1.0 / F,
                "sq1",
            )

            # transposes -> tpq [128=(4l,32slot), 8Q*72], block-diag bd
            tpq = work.tile([128, NQRT * NPOS], F16, tag="tpq")
            for Q in range(NQRT):
                tp = tps.tile([128, 128], F32, tag="tp")
                nc.tensor.transpose(
                    tp[:, 0:NPOS], out0_pad[:, Q * 128:(Q + 1) * 128], eye72[:]
                )
                nc.scalar.copy(tpq[:, Q * NPOS:(Q + 1) * NPOS], tp[:, 0:NPOS])
            bd = work.tile([128, NQRT * QBD], F16, tag="bd")
            nc.vector.memset(bd[:], 0.0)
            for Q in range(NQRT):
                for l in range(4):
                    nc.vector.tensor_copy(
                        bd[32 * l:32 * l + 32,
                           Q * QBD + l * NPOS:Q * QBD + (l + 1) * NPOS],
                        tpq[32 * l:32 * l + 32, Q * NPOS:(Q + 1) * NPOS],
                    )

            # ---------------- V + VP + agr + exp (software-pipelined) ----------------
            # e[c; (Q,cb,l,pos)] = exp(agr), SBUF fp16
            e = work.tile([128, NQRT * CBLK * QBD], F16, tag="e")

            def ired_phase(Q, VP):
                # agr = sum_i VP (identity-matmul accumulation), then exp
                for cb in range(CBLK):
                    agp = acc.tile([128, QBD], F32, tag="mm")
                    for i in range(DI):
                        t = i * CBLK + cb
                        nc.tensor.matmul(
                            agp[:],
                            eye128h[:],
                            VP[:, t * QBD:(t + 1) * QBD],
                            start=(i == 0),
                            stop=(i == DI - 1),
                        )
                    nc.scalar.activation(
                        e[:, (Q * CBLK + cb) * QBD:(Q * CBLK + cb + 1) * QBD],
                        agp[:],
                        AF.Exp,
                    )

            VPs = []
            for Q in range(NQRT):
                VP = ring.tile([128, NCHUNK * QBD], F16, tag="VP")
                for m in range(NCHUNK // 2):
                    t = 2 * m
                    # pair of V matmuls into one 2-bank PSUM tile (512-strided)
                    vh = vps.tile([128, 1024], F32, tag="vh")
                    for k in range(2):
                        nc.tensor.matmul(
                            vh[:, k * 512:k * 512 + QBD],
                            w8[:, (Q * NCHUNK + t + k) * 128:
                               (Q * NCHUNK + t + k + 1) * 128],
                            bd[:, Q * QBD:(Q + 1) * QBD],
                            start=True,
                            stop=True,
                        )
                    # VP = V * P (P broadcast over the 4 f's of the quartet)
                    pbp = (
                        p_ct[:, t * NPOS:(t + 2) * NPOS]
                        .rearrange("p (k n) -> p k n", k=2)
                        .unsqueeze(2)
                        .broadcast_to((128, 2, 4, NPOS))
                    )
                    vp_dst = VP[:, t * QBD:(t + 2) * QBD].rearrange(
                        "p (k j n) -> p k j n", k=2, j=4
                    )
                    if m % 3 == 2:
                        nc.vector.tensor_mul(
                            vp_dst,
                            vh[:].rearrange("p (k s) -> p k s", k=2)[:, :, 0:QBD]
                            .rearrange("p k (j n) -> p k j n", j=4),
                            pbp,
                        )
                    else:
                        vs = vsr.tile([128, 2 * QBD], F16, tag="vs")
                        nc.scalar.copy(
                            vs[:].rearrange("p (k s) -> p k s", k=2),
                            vh[:].rearrange("p (k s) -> p k s", k=2)[:, :, 0:QBD],
                        )
                        nc.vector.tensor_mul(
                            vp_dst,
                            vs[:].rearrange("p (k j n) -> p k j n", k=2, j=4),
                            pbp,
                        )
                VPs.append(VP)
                if Q > 0:
                    ired_phase(Q - 1, VPs[Q - 1])
            ired_phase(NQRT - 1, VPs[NQRT - 1])

            # ---------------- softmax normalizer + pp ----------------
            CN = CBLK * NPOS
            Zp = work.tile([128, NQRT * CN], F32, tag="Zp")
            for Q in range(NQRT):
                eq = e[:, Q * CBLK * QBD:(Q + 1) * CBLK * QBD].rearrange(
                    "p (cb l n) -> p cb l n", cb=CBLK, l=4
                )
                za = work.tile([128, CN], F32, tag="za")
                nc.vector.tensor_add(
                    za[:].rearrange("p (cb n) -> p cb n", cb=CBLK),
                    eq[:, :, 0, :], eq[:, :, 1, :],
                )
                zb = work.tile([128, CN], F32, tag="zb")
                nc.vector.tensor_add(
                    zb[:].rearrange("p (cb n) -> p cb n", cb=CBLK),
                    eq[:, :, 2, :], eq[:, :, 3, :],
                )
                nc.vector.tensor_add(
                    Zp[:, Q * CN:(Q + 1) * CN], za[:], zb[:]
                )
            Z4 = work.tile([128, 4 * CN], F32, tag="Z4")
            for m in range(4):
                nc.vector.tensor_add(
                    Z4[:, m * CN:(m + 1) * CN],
                    Zp[:, 2 * m * CN:(2 * m + 1) * CN],
                    Zp[:, (2 * m + 1) * CN:(2 * m + 2) * CN],
                )
            Z2 = work.tile([128, 2 * CN], F32, tag="Z2")
            for m in range(2):
                nc.vector.tensor_add(
                    Z2[:, m * CN:(m + 1) * CN],
                    Z4[:, 2 * m * CN:(2 * m + 1) * CN],
                    Z4[:, (2 * m + 1) * CN:(2 * m + 2) * CN],
                )
            Zf = work.tile([128, CN], F32, tag="Zf")
            nc.vector.tensor_add(Zf[:], Z2[:, 0:CN], Z2[:, CN:2 * CN])
            Zr = work.tile([128, CN], F32, tag="Zr")
            nc.vector.reciprocal(Zr[:], Zf[:])
            pp = work.tile([128, NCHUNK * NPOS], F16, tag="pp")
            nc.vector.tensor_mul(
                pp[:].rearrange("p (i cb n) -> p i cb n", i=DI, cb=CBLK),
                p_ct[:].rearrange("p (i cb n) -> p i cb n", i=DI, cb=CBLK),
                Zr[:].rearrange("p (cb n) -> p cb n", cb=CBLK)
                .unsqueeze(1)
                .broadcast_to((128, DI, CBLK, NPOS)),
            )

            # ---------------- S + centroids ----------------
            cen_sb = work.tile([128, 8 * NPOS], F32, tag="cen_sb")
            opre = work.tile([NPOS, 8 * 128], F32, tag="opre")
            for g in range(8):
                cg = cenp.tile([128, NPOS], F32, tag="cen")
                Ss = []
                for j in range(4):
                    S = ring.tile([128, NCHUNK * NPOS], F16, tag=f"S{j}")
                    eb = (
                        e[:, g * CBLK * QBD:(g + 1) * CBLK * QBD]
                        .rearrange("p (cb l n) -> p cb l n", cb=CBLK, l=4)
                        [:, :, j, :]
                        .unsqueeze(1)
                        .broadcast_to((128, DI, CBLK, NPOS))
                    )
                    nc.vector.tensor_mul(
                        S[:].rearrange("p (i cb n) -> p i cb n", i=DI, cb=CBLK),
                        pp[:].rearrange("p (i cb n) -> p i cb n", i=DI, cb=CBLK),
                        eb,
                    )
                    Ss.append(S)
                for t in range(NCHUNK):
                    for j in range(4):
                        f = 4 * g + j
                        nc.tensor.matmul(
                            cg[32 * j:32 * j + DO, :],
                            w_r[:, t * F * DO + f * DO:t * F * DO + (f + 1) * DO],
                            Ss[j][:, t * NPOS:(t + 1) * NPOS],
                            start=(t == 0),
                            stop=(t == NCHUNK - 1),
                            tile_position=(0, 32 * j),
                        )
                nc.scalar.copy(cen_sb[:, g * NPOS:(g + 1) * NPOS], cg[:])
                tp2 = tps.tile([128, 128], F32, tag="tp")
                nc.tensor.transpose(
                    tp2[0:NPOS, :], cen_sb[:, g * NPOS:(g + 1) * NPOS], eye128f[:]
                )
                nc.scalar.copy(opre[:, g * 128:(g + 1) * 128], tp2[0:NPOS, :])

            # ---------------- squash2 + output ----------------
            y_sb = work.tile([NPOS, F * DO], F32, tag="y_sb")
            squash(
                opre[:].rearrange("p (g j s) -> p g j s", g=8, j=4)[:, :, :, 0:DO],
                y_sb[:].rearrange("p (f o) -> p f o", o=DO),
                1.0,
                "sq2",
            )
            nc.sync.dma_start(y_d[:], y_sb[:])

    nc.compile()
    return nc


def _get_program():
    if "nc" not in _CACHE:
        _CACHE["nc"] = _build()
    return _CACHE["nc"]


def _in_maps(x, Wm):
    w_r, w8 = _host_weights(Wm)
    eye72 = np.eye(NPOS, dtype=np.float32)
    eye128h = np.eye(128, dtype=np.float16)
    eye128f = np.eye(128, dtype=np.float32)
    return [{
        "p_ct": _host_patches(x, k),
        "w_r": w_r,
        "w8": w8,
        "eye72": eye72,
        "eye128h": eye128h,
        "eye128f": eye128f,
    } for k in range(8)]


def kernel(x, W):
    from concourse.bass_utils import run_bass_kernel_spmd

    x = np.asarray(x, np.float32)
    Wm = np.asarray(W, np.float32)[0, 0, 0]
    nc = _get_program()
    res = run_bass_kernel_spmd(nc, _in_maps(x, Wm), list(range(8)))
    Ho, Wo = H - KH + 1, WD - KW + 1
    y = np.empty((B, Ho, Wo, F, DO), np.float32)
    for k in range(8):
        b, hh = divmod(k, 2)
        y[b, 6 * hh:6 * hh + 6] = res.results[k]["y"].reshape(6, Wo, F, DO)
    return y
